# revision 1
# baseline (speedup 1.0000x reference)
"""GCN forward kernel for 8 Trainium2 NeuronCores (Bass/Tile).

    h   = BN1(leaky_relu(x @ W1 + b1))
    h2  = BN2(leaky_relu(gcn_conv(h @ Wc) + bc))
    out = log_softmax(concat(h, h2) @ W2 + b2)

Nodes are sharded over 8 cores. The GCN scatter/gather runs on-device via
SWDGE dma_gather / dma_scatter_add; BN statistics use AllReduce and the
conv's neighbor features use AllGather. BN1's affine transform is folded
into Wc/W2a (plus a per-destination norm-sum correction term) so the h
tensor is only materialized once, pre-BN.

Self-contained: builds the Bass program from the actual inputs each call,
runs SPMD on cores 0-7, reassembles the full output on the host.
"""

import sys

sys.path.insert(0, "/opt/trn_rl_repo")

import numpy as np
import concourse.bass as bass
import concourse.bacc as bacc
import concourse.mybir as mybir
import concourse.tile as tile
from concourse.bass_utils import run_bass_kernel_spmd
from concourse.masks import make_identity

NCORES = 8
EPS = 1e-5
SLOPE = 0.01
F32 = mybir.dt.float32
I16 = mybir.dt.int16
AF = mybir.ActivationFunctionType
OP = mybir.AluOpType

# ---------------------------------------------------------------------------
# this walrus build allows at most ONE sync-wait per instruction; spread
# extra waits over nops inserted before the instruction on the same engine.
_MAXW = 1


def _split_multi_waits(nc):
    for bb in nc.main_func.blocks:
        insts = bb.instructions
        i = 0
        while i < len(insts):
            inst = insts[i]
            si = inst.sync_info
            waits = list(si.on_wait) if si is not None else []
            if len(waits) > _MAXW:
                si.on_wait = waits[-_MAXW:]
                extra = waits[:-_MAXW]
                pos = i
                for j in range(0, len(extra), _MAXW):
                    nop = mybir.InstNoOp(
                        name=f"waitsplit-{nc.next_id()}",
                        sync_info=mybir.SyncInfo(
                            on_wait=extra[j : j + _MAXW], on_update=[]
                        ),
                        bass_nofuse=True,
                        engine=inst.engine,
                    )
                    insts.insert(pos, nop)
                    pos += 1
                    i += 1
            i += 1


def _finish(nc):
    nc.compile()
    _split_multi_waits(nc)
    bass.Bass.finalize(nc)


# ---------------------------------------------------------------------------
# host-side graph preprocessing


def _preprocess(N, edge_index, edge_weight):
    NSH = N // NCORES
    G = (NSH + 127) // 128
    NPAD = G * 128

    row = np.asarray(edge_index[0], dtype=np.int64)
    col = np.asarray(edge_index[1], dtype=np.int64)
    w = np.asarray(edge_weight, dtype=np.float64)

    deg = np.bincount(col, weights=w, minlength=N) + 1.0
    dinv = 1.0 / np.sqrt(deg)
    norm = (dinv[row] * w * dinv[col]).astype(np.float32)

    loops = np.arange(N, dtype=np.int64)
    rows_all = np.concatenate([row, loops])
    cols_all = np.concatenate([col, loops])
    norms_all = np.concatenate([norm, (dinv * dinv).astype(np.float32)])

    shard = rows_all // NSH
    rpad = shard * NPAD + (rows_all - shard * NSH)
    blk = rpad // 4
    ppos = rpad % 2
    pss = (rpad % 4) // 2

    dest_core = cols_all // NSH
    dest_loc = cols_all - dest_core * NSH

    per = {}
    meta = {}
    for c in range(NCORES):
        in_c = dest_core == c
        for p in range(2):
            m = in_c & (pss == p)
            dl = dest_loc[m]
            per[(c, p)] = (dl, blk[m], ppos[m], norms_all[m])
            d = np.bincount(dl, minlength=NSH)
            perm = np.argsort(-d, kind="stable")
            meta[(c, p)] = (perm, d[perm][::128][:G].copy(), d)

    sched = []
    for p in range(2):
        s = np.zeros(G, dtype=np.int64)
        for c in range(NCORES):
            s = np.maximum(s, meta[(c, p)][1])
        sched.append(s)
    S = int(max(sched[0].sum(), sched[1].sum()))
    S = max(S, 1)

    offs = []
    for p in range(2):
        o = np.zeros(G + 1, dtype=np.int64)
        o[1:] = np.cumsum(sched[p])
        offs.append(o)

    CH = int(max(32, sched[0].max(), sched[1].max()))
    assert CH <= 64, f"group degree {CH} exceeds gather chunk limit"
    chunks = []
    for p in range(2):
        cl = []
        cur = None
        for g in range(G):
            d = int(sched[p][g])
            if d == 0:
                continue
            if cur is None or cur[1] + d > CH:
                cur = [int(offs[p][g]), 0, []]
                cl.append(cur)
            cur[2].append((g, cur[1], d))
            cur[1] += d
        chunks.append(cl)

    def wrap16(flat):
        return np.tile(flat.reshape(-1, 16).T.copy(), (8, 1))

    arrs = {}
    for c in range(NCORES):
        core = {}
        for p in range(2):
            dl, bk, pp, nm = per[(c, p)]
            perm, gmax, d = meta[(c, p)]
            inv = np.empty(NSH, dtype=np.int64)
            inv[perm] = np.arange(NSH)
            gp = inv[dl]
            order = np.argsort(gp, kind="stable")
            bk_s, pp_s, nm_s = bk[order], pp[order], nm[order]
            gp_s = gp[order]
            grow = gp_s % 128
            ggrp = gp_s // 128
            first = np.r_[True, gp_s[1:] != gp_s[:-1]]
            idx_first = np.flatnonzero(first)
            runlen = np.diff(np.r_[idx_first, len(gp_s)])
            rank = np.arange(len(gp_s)) - np.repeat(idx_first, runlen)
            scol = offs[p][ggrp] + rank
            assert (rank < sched[p][ggrp]).all()

            idx_flat = np.zeros(S * 128, dtype=np.int16)
            idx_flat[scol * 128 + grow] = bk_s.astype(np.int16)
            nl = np.zeros((128, S, 2), dtype=np.float32)
            nl[grow, scol, pp_s] = nm_s

            sc = np.full(NPAD, -1, dtype=np.int16)
            sc[:NSH] = perm.astype(np.int16)

            core[f"eidx{p}"] = wrap16(idx_flat)
            core[f"enrm{p}"] = nl.reshape(128, S * 2)
            core[f"sidx{p}"] = wrap16(sc)
        arrs[c] = core

    return dict(N=N, NSH=NSH, G=G, NPAD=NPAD, S=S, CH=CH, chunks=chunks,
                offs=offs, sched=sched, arrs=arrs)


# ---------------------------------------------------------------------------
# device program (SPMD; identical on every core)


def _build(meta, F_IN, H1, H2, NC_):
    N, NSH, G, NPAD, S, CH = (meta[k] for k in ("N", "NSH", "G", "NPAD", "S", "CH"))
    chunks = meta["chunks"]
    KC = F_IN // 128
    PW = H2 + 1        # PART row width: 32 feature sums + norm-sum

    nc = bacc.Bacc("TRN2", target_bir_lowering=False, debug=False,
                   num_devices=NCORES, dynamic_dma_scratch_size=65536,
                   num_swdge_queues=4)

    x_in = nc.declare_dram_parameter("x", [NPAD, F_IN], F32, isOutput=False)
    w1_in = nc.declare_dram_parameter("W1", [F_IN, H1], F32, isOutput=False)
    b1_in = nc.declare_dram_parameter("b1", [H1, 1], F32, isOutput=False)
    g1_in = nc.declare_dram_parameter("g1", [H1, 1], F32, isOutput=False)
    be1_in = nc.declare_dram_parameter("be1", [H1, 1], F32, isOutput=False)
    wc_in = nc.declare_dram_parameter("Wc", [H1, H2], F32, isOutput=False)
    bc_in = nc.declare_dram_parameter("bc", [1, H2], F32, isOutput=False)
    g2_in = nc.declare_dram_parameter("g2c", [H2, 1], F32, isOutput=False)
    be2_in = nc.declare_dram_parameter("be2c", [H2, 1], F32, isOutput=False)
    w2_in = nc.declare_dram_parameter("W2", [H1 + H2, NC_], F32, isOutput=False)
    b2_in = nc.declare_dram_parameter("b2", [1, NC_], F32, isOutput=False)
    ei = [nc.declare_dram_parameter(f"eidx{p}", [128, S * 8], I16, isOutput=False)
          for p in range(2)]
    en = [nc.declare_dram_parameter(f"enrm{p}", [128, S * 2], F32, isOutput=False)
          for p in range(2)]
    si = [nc.declare_dram_parameter(f"sidx{p}", [128, NPAD // 16], I16, isOutput=False)
          for p in range(2)]
    out_t = nc.declare_dram_parameter("out", [NPAD, NC_], F32, isOutput=True)

    xl_local = nc.dram_tensor("xl_local", [NPAD, H2], F32)
    xl_full = nc.dram_tensor("xl_full", [NPAD * NCORES, H2], F32, addr_space="Shared")
    hT_d = nc.dram_tensor("hT_d", [H1, NPAD], F32)
    h2acc = nc.dram_tensor("h2acc", [NPAD, 2 * H2], F32)
    bn1_i = nc.dram_tensor("bn1_i", [H1, 2], F32)
    bn1_o = nc.dram_tensor("bn1_o", [H1, 2], F32, addr_space="Shared")
    bn2_i = nc.dram_tensor("bn2_i", [1, 2 * H2], F32)
    bn2_o = nc.dram_tensor("bn2_o", [1, 2 * H2], F32, addr_space="Shared")

    rg = [list(range(NCORES))]
    RLAST = NSH - (G - 1) * 128      # valid rows in last tile

    with tile.TileContext(nc) as tc:
        with (
            tc.tile_pool(name="pers", bufs=1) as pers,
            tc.tile_pool(name="work", bufs=3) as work,
            tc.tile_pool(name="mpool", bufs=1) as mpool,
            tc.tile_pool(name="psc", bufs=2, space="PSUM") as psc,
            tc.tile_pool(name="psp", bufs=1, space="PSUM") as psp,
        ):
            ident = pers.tile([128, 128], F32, tag="ident")
            make_identity(nc, ident[:])
            ones_c = pers.tile([128, 1], F32, tag="ones_c")
            nc.gpsimd.memset(ones_c[:], 1.0)
            ones_r = pers.tile([1, 128], F32, tag="ones_r")
            nc.gpsimd.memset(ones_r[:], 1.0)

            # zero h2acc early
            zt = pers.tile([128, 2 * H2], F32, tag="zt")
            nc.gpsimd.memset(zt[:], 0.0)
            for t in range(G):
                nc.sync.dma_start(out=h2acc[t * 128:(t + 1) * 128, :], in_=zt[:])

            # ---------------- stage 1: h = leaky(x@W1 + b1) -> hT_d + stats
            w1_sb = pers.tile([128, KC * H1], F32, tag="w1")
            nc.sync.dma_start(
                out=w1_sb[:].rearrange("p (k m) -> p k m", k=KC),
                in_=w1_in[:].rearrange("(k p) m -> p k m", p=128),
            )
            b1_sb = pers.tile([H1, 1], F32, tag="b1")
            nc.sync.dma_start(out=b1_sb[:], in_=b1_in[:])
            s1 = pers.tile([H1, 1], F32, tag="s1")
            nc.gpsimd.memset(s1[:], 0.0)
            sq1 = pers.tile([H1, 1], F32, tag="sq1")
            nc.gpsimd.memset(sq1[:], 0.0)

            for t in range(G):
                R = RLAST if t == G - 1 else 128
                xt = work.tile([128, F_IN], F32, tag="xt")
                nc.sync.dma_start(out=xt[:], in_=x_in[t * 128:(t + 1) * 128, :])
                xT = work.tile([128, F_IN], F32, tag="xT")
                for k in range(KC):
                    tp = psc.tile([128, 128], F32, tag="tp")
                    nc.tensor.transpose(out=tp[:], in_=xt[:, k * 128:(k + 1) * 128],
                                        identity=ident[:])
                    if k % 2 == 0:
                        nc.vector.tensor_copy(out=xT[:, k * 128:(k + 1) * 128], in_=tp[:])
                    else:
                        nc.scalar.copy(out=xT[:, k * 128:(k + 1) * 128], in_=tp[:])
                hp = psc.tile([H1, 128], F32, tag="mm")
                for k in range(KC):
                    nc.tensor.matmul(out=hp[:], lhsT=w1_sb[:, k * H1:(k + 1) * H1],
                                     rhs=xT[:, k * 128:(k + 1) * 128],
                                     start=(k == 0), stop=(k == KC - 1))
                hsl = work.tile([H1, 128], F32, tag="hsl")
                nc.scalar.activation(out=hsl[:], in_=hp[:], func=AF.Identity,
                                     bias=b1_sb[:])
                nc.vector.scalar_tensor_tensor(out=hsl[:], in0=hsl[:], scalar=SLOPE,
                                               in1=hsl[:], op0=OP.mult, op1=OP.max)
                nc.sync.dma_start(out=hT_d[:, t * 128:(t + 1) * 128], in_=hsl[:])
                # stats over valid rows
                st = work.tile([H1, 1], F32, tag="st")
                nc.vector.tensor_reduce(out=st[:], in_=hsl[:, :R],
                                        axis=mybir.AxisListType.X, op=OP.add)
                nc.vector.tensor_tensor(out=s1[:], in0=s1[:], in1=st[:], op=OP.add)
                sqs = work.tile([H1, 128], F32, tag="sqs")
                sqt = work.tile([H1, 1], F32, tag="sqt")
                nc.scalar.activation(out=sqs[:, :R], in_=hsl[:, :R], func=AF.Square,
                                     accum_out=sqt[:])
                nc.vector.tensor_tensor(out=sq1[:], in0=sq1[:], in1=sqt[:], op=OP.add)

            # ---------------- BN1 stats allreduce -> fold into Wc', W2a'
            st1 = pers.tile([H1, 2], F32, tag="st1")
            nc.vector.tensor_copy(out=st1[:, 0:1], in_=s1[:])
            nc.vector.tensor_copy(out=st1[:, 1:2], in_=sq1[:])
            nc.sync.dma_start(out=bn1_i[:], in_=st1[:])
            nc.gpsimd.collective_compute("AllReduce", OP.add, replica_groups=rg,
                                         ins=[bn1_i[:]], outs=[bn1_o[:]])
            sr1 = pers.tile([H1, 2], F32, tag="sr1")
            nc.sync.dma_start(out=sr1[:], in_=bn1_o[:])
            mean1 = pers.tile([H1, 1], F32, tag="mean1")
            nc.scalar.mul(mean1[:], sr1[:, 0:1], 1.0 / N)
            var1 = pers.tile([H1, 1], F32, tag="var1")
            nc.scalar.mul(var1[:], sr1[:, 1:2], 1.0 / N)
            tmp1 = pers.tile([H1, 1], F32, tag="tmp1")
            nc.vector.tensor_tensor(out=tmp1[:], in0=mean1[:], in1=mean1[:], op=OP.mult)
            nc.vector.tensor_tensor(out=var1[:], in0=var1[:], in1=tmp1[:], op=OP.subtract)
            nc.vector.tensor_scalar_add(var1[:], var1[:], EPS)
            sd1 = pers.tile([H1, 1], F32, tag="sd1")
            nc.scalar.activation(out=sd1[:], in_=var1[:], func=AF.Sqrt)
            inv1 = pers.tile([H1, 1], F32, tag="inv1")
            nc.vector.reciprocal(out=inv1[:], in_=sd1[:])
            g1_sb = pers.tile([H1, 1], F32, tag="g1s")
            nc.sync.dma_start(out=g1_sb[:], in_=g1_in[:])
            be1_sb = pers.tile([H1, 1], F32, tag="be1s")
            nc.sync.dma_start(out=be1_sb[:], in_=be1_in[:])
            sc1 = pers.tile([H1, 1], F32, tag="sc1")
            nc.vector.tensor_tensor(out=sc1[:], in0=inv1[:], in1=g1_sb[:], op=OP.mult)
            bi1 = pers.tile([H1, 1], F32, tag="bi1")
            nc.vector.tensor_tensor(out=bi1[:], in0=mean1[:], in1=sc1[:], op=OP.mult)
            nc.vector.tensor_tensor(out=bi1[:], in0=be1_sb[:], in1=bi1[:],
                                    op=OP.subtract)

            wc_sb = pers.tile([H1, H2], F32, tag="wc")
            nc.sync.dma_start(out=wc_sb[:], in_=wc_in[:])
            wcp = pers.tile([H1, H2], F32, tag="wcp")      # diag(sc1) @ Wc
            nc.vector.tensor_scalar_mul(wcp[:], wc_sb[:], sc1[:])
            qp = psc.tile([1, H2], F32, tag="bc")          # q = bi1 @ Wc
            nc.tensor.matmul(out=qp[:], lhsT=bi1[:], rhs=wc_sb[:], start=True, stop=True)
            q_sb = pers.tile([1, H2], F32, tag="q")
            nc.vector.tensor_copy(out=q_sb[:], in_=qp[:])
            qbp = psc.tile([128, H2], F32, tag="bc")
            nc.tensor.matmul(out=qbp[:], lhsT=ones_r[:], rhs=q_sb[:], start=True,
                             stop=True)
            q_b = pers.tile([128, H2], F32, tag="q_b")
            nc.vector.tensor_copy(out=q_b[:], in_=qbp[:])

            # ---------------- xl = h @ Wc' -> allgather
            for t in range(G):
                hbt = work.tile([H1, 128], F32, tag="hbt")
                nc.sync.dma_start(out=hbt[:], in_=hT_d[:, t * 128:(t + 1) * 128])
                xp = psc.tile([128, H2], F32, tag="mm")
                nc.tensor.matmul(out=xp[:], lhsT=hbt[:], rhs=wcp[:], start=True,
                                 stop=True)
                xs = work.tile([128, H2], F32, tag="xs")
                nc.scalar.copy(out=xs[:], in_=xp[:])
                nc.sync.dma_start(out=xl_local[t * 128:(t + 1) * 128, :], in_=xs[:])
            nc.gpsimd.collective_compute("AllGather", OP.bypass, replica_groups=rg,
                                         ins=[xl_local[:]], outs=[xl_full[:]])

            # ---------------- edge aggregation: two quad-parity passes
            NBLK = NPAD * NCORES // 4
            for p in range(2):
                eidx_sb = pers.tile([128, S * 8], I16, tag="eidx")
                nc.sync.dma_start(out=eidx_sb[:], in_=ei[p][:])
                enrm_sb = pers.tile([128, S * 2], F32, tag="enrm")
                nc.sync.dma_start(out=enrm_sb[:], in_=en[p][:])
                part = pers.tile([128, G * PW], F32, tag="part")
                nc.vector.memset(part[:], 0.0)

                src_ap = bass.AP(tensor=xl_full, offset=p * 2 * H2,
                                 ap=[[4 * H2, NBLK], [1, 2 * H2]])
                for ci, (col0, cols, groups) in enumerate(chunks[p]):
                    qn = ci % 4
                    ms = mpool.tile([128, CH * 2 * H2], F32, tag=f"ms{qn}")
                    nc.gpsimd.dma_gather(
                        out_ap=ms[:, :cols * 2 * H2].rearrange(
                            "p (m e) -> p m e", e=2 * H2),
                        in_ap=src_ap,
                        idxs_ap=eidx_sb[:, col0 * 8:(col0 + cols) * 8],
                        num_idxs=cols * 128, num_idxs_reg=cols * 128,
                        elem_size=2 * H2, elem_step=4 * H2,
                        single_packet=False, queue_num=qn,
                    )
                    mv = ms[:, :cols * 2 * H2].rearrange(
                        "p (m h f) -> p m h f", h=2, f=H2)
                    nv = bass.AP(
                        tensor=enrm_sb[:].tensor,
                        offset=enrm_sb[:].offset + col0 * 2,
                        ap=[list(enrm_sb[:].ap[0]), [2, cols], [1, 2], [0, H2]],
                    )
                    nc.vector.tensor_tensor(out=mv, in0=mv, in1=nv, op=OP.mult)
                    for (g, rel, d) in groups:
                        rv = bass.AP(
                            tensor=ms[:].tensor,
                            offset=ms[:].offset + rel * 2 * H2,
                            ap=[list(ms[:].ap[0]), [1, H2], [32, 2 * d]],
                        )
                        nc.vector.tensor_reduce(
                            out=part[:, g * PW:g * PW + H2], in_=rv,
                            axis=mybir.AxisListType.X, op=OP.add)
                        sv = bass.AP(
                            tensor=enrm_sb[:].tensor,
                            offset=enrm_sb[:].offset + int(meta["offs"][p][g]) * 2,
                            ap=[list(enrm_sb[:].ap[0]), [1, 2 * d]],
                        )
                        nc.vector.tensor_reduce(
                            out=part[:, g * PW + H2:g * PW + PW], in_=sv,
                            axis=mybir.AxisListType.X, op=OP.add)

                sidx_sb = pers.tile([128, NPAD // 16], I16, tag="sidx")
                nc.sync.dma_start(out=sidx_sb[:], in_=si[p][:])
                out_ap = bass.AP(tensor=h2acc, offset=0,
                                 ap=[[2 * H2, NPAD], [1, PW]])
                nc.gpsimd.dma_scatter_add(
                    out_ap=out_ap,
                    in_ap=part[:].rearrange("p (m e) -> p m e", e=PW),
                    idxs_ap=sidx_sb[:],
                    num_idxs=NPAD, num_idxs_reg=NSH,
                    elem_size=PW, elem_step=2 * H2,
                    single_packet=False, queue_num=p,
                )

            # ---------------- h2 = leaky(h2acc + nsum*q + bc); BN2 stats
            bc_sb = pers.tile([1, H2], F32, tag="bcr")
            nc.sync.dma_start(out=bc_sb[:], in_=bc_in[:])
            bcp = psc.tile([128, H2], F32, tag="bc")
            nc.tensor.matmul(out=bcp[:], lhsT=ones_r[:], rhs=bc_sb[:], start=True,
                             stop=True)
            bcb = pers.tile([128, H2], F32, tag="bcb")
            nc.vector.tensor_copy(out=bcb[:], in_=bcp[:])

            h2_sb = pers.tile([128, G * H2], F32, tag="h2")
            st2p = psp.tile([1, H2], F32, tag="st2p")
            sq2p = psp.tile([1, H2], F32, tag="sq2p")
            for t in range(G):
                R = RLAST if t == G - 1 else 128
                ht = work.tile([128, 2 * H2], F32, tag="ht")
                nc.sync.dma_start(out=ht[:], in_=h2acc[t * 128:(t + 1) * 128, :])
                sl = h2_sb[:, t * H2:(t + 1) * H2]
                if R < 128:
                    nc.gpsimd.memset(sl, 0.0)
                slr = h2_sb[:R, t * H2:(t + 1) * H2]
                nc.vector.scalar_tensor_tensor(out=slr, in0=q_b[:R, :],
                                               scalar=ht[:R, H2:H2 + 1],
                                               in1=ht[:R, :H2],
                                               op0=OP.mult, op1=OP.add)
                nc.vector.tensor_tensor(out=slr, in0=slr, in1=bcb[:R, :], op=OP.add)
                nc.vector.scalar_tensor_tensor(out=slr, in0=slr, scalar=SLOPE,
                                               in1=slr, op0=OP.mult, op1=OP.max)
                nc.tensor.matmul(out=st2p[:], lhsT=ones_c[:], rhs=sl,
                                 start=(t == 0), stop=(t == G - 1))
                sqs2 = work.tile([128, H2], F32, tag="sqs2")
                nc.scalar.activation(out=sqs2[:], in_=sl, func=AF.Square)
                nc.tensor.matmul(out=sq2p[:], lhsT=ones_c[:], rhs=sqs2[:],
                                 start=(t == 0), stop=(t == G - 1))

            st2 = pers.tile([1, 2 * H2], F32, tag="st2")
            nc.vector.tensor_copy(out=st2[:, :H2], in_=st2p[:])
            nc.vector.tensor_copy(out=st2[:, H2:], in_=sq2p[:])
            nc.sync.dma_start(out=bn2_i[:], in_=st2[:])
            nc.gpsimd.collective_compute("AllReduce", OP.add, replica_groups=rg,
                                         ins=[bn2_i[:]], outs=[bn2_o[:]])
            sr2 = pers.tile([1, 2 * H2], F32, tag="sr2")
            nc.sync.dma_start(out=sr2[:], in_=bn2_o[:])
            # transpose to column form [2*H2, 1]
            sr2tp = psc.tile([2 * H2, 1], F32, tag="bc")
            nc.tensor.transpose(out=sr2tp[:], in_=sr2[:], identity=ident[:1, :1])
            sr2t = pers.tile([2 * H2, 1], F32, tag="sr2t")
            nc.vector.tensor_copy(out=sr2t[:], in_=sr2tp[:])
            mean2 = pers.tile([H2, 1], F32, tag="mean2")
            nc.scalar.mul(mean2[:], sr2t[:H2, :], 1.0 / N)
            var2 = pers.tile([H2, 1], F32, tag="var2")
            nc.scalar.mul(var2[:], sr2t[H2:, :], 1.0 / N)
            tmp2 = pers.tile([H2, 1], F32, tag="tmp2")
            nc.vector.tensor_tensor(out=tmp2[:], in0=mean2[:], in1=mean2[:], op=OP.mult)
            nc.vector.tensor_tensor(out=var2[:], in0=var2[:], in1=tmp2[:],
                                    op=OP.subtract)
            nc.vector.tensor_scalar_add(var2[:], var2[:], EPS)
            sd2 = pers.tile([H2, 1], F32, tag="sd2")
            nc.scalar.activation(out=sd2[:], in_=var2[:], func=AF.Sqrt)
            inv2 = pers.tile([H2, 1], F32, tag="inv2")
            nc.vector.reciprocal(out=inv2[:], in_=sd2[:])
            g2_sb = pers.tile([H2, 1], F32, tag="g2s")
            nc.sync.dma_start(out=g2_sb[:], in_=g2_in[:])
            be2_sb = pers.tile([H2, 1], F32, tag="be2s")
            nc.sync.dma_start(out=be2_sb[:], in_=be2_in[:])
            sc2 = pers.tile([H2, 1], F32, tag="sc2")
            nc.vector.tensor_tensor(out=sc2[:], in0=inv2[:], in1=g2_sb[:], op=OP.mult)
            bi2 = pers.tile([H2, 1], F32, tag="bi2")
            nc.vector.tensor_tensor(out=bi2[:], in0=mean2[:], in1=sc2[:], op=OP.mult)
            nc.vector.tensor_tensor(out=bi2[:], in0=be2_sb[:], in1=bi2[:],
                                    op=OP.subtract)

            # fold: W2a' = diag(sc1)W2a, W2b' = diag(sc2)W2b,
            # c0 = b2 + bi1@W2a + bi2@W2b
            w2a_sb = pers.tile([H1, NC_], F32, tag="w2a")
            nc.sync.dma_start(out=w2a_sb[:], in_=w2_in[:H1, :])
            w2b_sb = pers.tile([H2, NC_], F32, tag="w2b")
            nc.sync.dma_start(out=w2b_sb[:], in_=w2_in[H1:, :])
            w2ap = pers.tile([H1, NC_], F32, tag="w2ap")
            nc.vector.tensor_scalar_mul(w2ap[:], w2a_sb[:], sc1[:])
            w2bp = pers.tile([H2, NC_], F32, tag="w2bp")
            nc.vector.tensor_scalar_mul(w2bp[:], w2b_sb[:], sc2[:])
            c0p = psc.tile([1, NC_], F32, tag="bc")
            nc.tensor.matmul(out=c0p[:], lhsT=bi1[:], rhs=w2a_sb[:], start=True,
                             stop=False)
            nc.tensor.matmul(out=c0p[:], lhsT=bi2[:], rhs=w2b_sb[:], start=False,
                             stop=True)
            b2_sb = pers.tile([1, NC_], F32, tag="b2")
            nc.sync.dma_start(out=b2_sb[:], in_=b2_in[:])
            c0 = pers.tile([1, NC_], F32, tag="c0")
            nc.vector.tensor_tensor(out=c0[:], in0=c0p[:], in1=b2_sb[:], op=OP.add)
            c0bp = psc.tile([128, NC_], F32, tag="bc")
            nc.tensor.matmul(out=c0bp[:], lhsT=ones_r[:], rhs=c0[:], start=True,
                             stop=True)
            c0b = pers.tile([128, NC_], F32, tag="c0b")
            nc.vector.tensor_copy(out=c0b[:], in_=c0bp[:])

            # ---------------- logits + log_softmax
            for t in range(G):
                hbt = work.tile([H1, 128], F32, tag="hbt")
                nc.sync.dma_start(out=hbt[:], in_=hT_d[:, t * 128:(t + 1) * 128])
                lg = psc.tile([128, NC_], F32, tag="mm")
                nc.tensor.matmul(out=lg[:], lhsT=hbt[:], rhs=w2ap[:], start=True,
                                 stop=False)
                h2tp = psc.tile([H2, 128], F32, tag="tp")
                nc.tensor.transpose(out=h2tp[:], in_=h2_sb[:, t * H2:(t + 1) * H2],
                                    identity=ident[:])
                h2t = work.tile([H2, 128], F32, tag="h2t")
                nc.vector.tensor_copy(out=h2t[:], in_=h2tp[:])
                nc.tensor.matmul(out=lg[:], lhsT=h2t[:], rhs=w2bp[:], start=False,
                                 stop=True)
                lgs = work.tile([128, NC_], F32, tag="lgs")
                nc.vector.tensor_tensor(out=lgs[:], in0=lg[:], in1=c0b[:], op=OP.add)
                mx = work.tile([128, 1], F32, tag="mx")
                nc.vector.tensor_reduce(out=mx[:], in_=lgs[:],
                                        axis=mybir.AxisListType.X, op=OP.max)
                xm = work.tile([128, NC_], F32, tag="xm")
                nc.vector.tensor_scalar_sub(xm[:], lgs[:], mx[:])
                ex = work.tile([128, NC_], F32, tag="ex")
                se = work.tile([128, 1], F32, tag="se")
                nc.scalar.activation(out=ex[:], in_=xm[:], func=AF.Exp,
                                     accum_out=se[:])
                ls = work.tile([128, 1], F32, tag="ls")
                nc.scalar.activation(out=ls[:], in_=se[:], func=AF.Ln)
                fin = work.tile([128, NC_], F32, tag="fin")
                nc.vector.tensor_scalar_sub(fin[:], xm[:], ls[:])
                nc.sync.dma_start(out=out_t[t * 128:(t + 1) * 128, :], in_=fin[:])

    _finish(nc)
    return nc


# ---------------------------------------------------------------------------


def kernel(x, edge_index, edge_weight, W1, b1, g1, be1, Wc, bc, g2, be2, W2, b2):
    x = np.asarray(x, dtype=np.float32)
    N, F_IN = x.shape
    H1 = np.asarray(W1).shape[1]
    H2 = np.asarray(Wc).shape[1]
    NC_ = np.asarray(W2).shape[1]

    meta = _preprocess(N, edge_index, edge_weight)
    NSH, NPAD = meta["NSH"], meta["NPAD"]

    nc = _build(meta, F_IN, H1, H2, NC_)

    in_maps = []
    for c in range(NCORES):
        xs = np.zeros((NPAD, F_IN), np.float32)
        xs[:NSH] = x[c * NSH:(c + 1) * NSH]
        m = {
            "x": xs,
            "W1": np.asarray(W1, np.float32),
            "b1": np.asarray(b1, np.float32).reshape(-1, 1),
            "g1": np.asarray(g1, np.float32).reshape(-1, 1),
            "be1": np.asarray(be1, np.float32).reshape(-1, 1),
            "Wc": np.asarray(Wc, np.float32),
            "bc": np.asarray(bc, np.float32).reshape(1, -1),
            "g2c": np.asarray(g2, np.float32).reshape(-1, 1),
            "be2c": np.asarray(be2, np.float32).reshape(-1, 1),
            "W2": np.asarray(W2, np.float32),
            "b2": np.asarray(b2, np.float32).reshape(1, -1),
        }
        m.update(meta["arrs"][c])
        in_maps.append(m)

    res = run_bass_kernel_spmd(nc, in_maps, list(range(NCORES)))
    return np.concatenate([res.results[c]["out"][:NSH] for c in range(NCORES)],
                          axis=0)

